# revision 36
# baseline (speedup 1.0000x reference)
"""Trainium2 Bass kernel for the capsule-routing layer.

Math (derived from the reference):
  u_hat[b,i,j,k] = sum_d x[b,j,d] W[d, i*32+k]   (never materialized!)
  iter t: c = softmax_i(b_logits); s[i,k] = sum_j c[i,j] u_hat[i,j,k]
          o = s / sqrt(sum_k s^2 + eps); b_logits[i,j] = sum_k o[i,k] u_hat[i,j,k]
Substituting u_hat = x @ W everywhere:
  y[i,d]    = sum_j c[i,j] x[j,d]             (small matmul, K=1024)
  s[i,k]    = sum_d y[i,d] W[d, i*32+k]       (block-diagonal of y @ W)
  wt[d,i]   = sum_k W[d, i*32+k] s[i,k]       (W @ block-diag(s); squash scale
                                               folded into the Exp that follows)
  b[i,j]    = sum_d x[j,d] wt[d,i]            (small matmul, K=256)
This removes the 34-GFLOP u_hat product entirely (~7.6x FLOP reduction).

W's flattened (i,k) axis is stored K-MAJOR on chip (q = k*32 + i), so the
final block-diagonal compaction s4c[p,k] = sum_i' masked[p, (k,i')] is a
CONTIGUOUS last-axis reduce (fast DVE path) instead of a strided one.

Sharding: data-parallel, 8 batches per core; batches processed in groups of
4 stacked on SBUF partitions (partition p = 32*b + i).

Schedule highlights vs the naive pipeline:
  - DMA: the SWDGE (gpsimd) queue hogs ~85% of HBM bandwidth once active,
    starving the two HWDGE rings. So the rings carry only what fully lands
    before SWDGE data starts flowing (group-0 xb chunks 0-1 + weights +
    32KB constants); everything else rides SWDGE in strict deadline order.
    First y-matmul fires ~10us in (vs ~22us when the whole 2MB xb tile was
    one transfer).
  - xd (the b-logit matmul's moving operand) is fp8_e4m3: it only steers
    routing, never touches the output path. Halves its DMA and lifts rel
    err 0.4% -> 0.8% (gate: 2%). fp8 rhs x bf16 lhsT matmuls work fine.
  - every mask is a broadcast AP view of one 32KB idT tile (idT[p,c] =
    (p%32==c%32)) instead of 512KB of DRAM masks.
  - O = maskT*(W^T y) is just m4 transposed, so 8 PE transposes replace a
    16-matmul recompute and two DVE mask-multiplies.
  - e4 is transposed with the PE (4 [128,128] transposes into PSUM) instead
    of the DMA xbar (~2us init latency); the softmax reduce and c-mult read
    the transposed tile straight out of PSUM. GPSIMD/Pool cannot touch
    PSUM, and free-axis tensor_reduce is DVE-only, so the softmax chain
    stays on DVE; Pool gets the SBUF-only Newton-rsqrt steps and group-0's
    tail so the final group's DVE chain isn't queued behind them.
  - group 0's last-iteration block is emitted 2 scheduler slices early:
    engine queues are in-order, and lockstep emission would park g0's tail
    ops behind g1's (time-wise lagging) softmax chain.
  - keeping the PE queue dense matters beyond occupancy: the tensor engine
    clock ramps 0.65 -> 1.2 -> 2.4 GHz with sustained execution and resets
    on stalls.
"""

import numpy as np

try:
    import concourse.bass as bass
except ImportError:  # path fallback for bare environments
    import sys

    sys.path.insert(0, "/opt/trn_rl_repo")
    import concourse.bass as bass

from contextlib import ExitStack

import concourse.bacc as bacc
import concourse.tile as tile
from concourse import mybir
from concourse.bass_utils import run_bass_kernel_spmd

F32 = mybir.dt.float32
BF16 = mybir.dt.bfloat16
F8 = mybir.dt.float8e4
AF = mybir.ActivationFunctionType
ALU = mybir.AluOpType

NUM_CAPS = 32
DIM_CAPS = 32
D_IN = 256  # feature dim (d)
N_IN = 1024  # input capsule count (j)
IK = NUM_CAPS * DIM_CAPS  # 1024 flattened (k,i) [K-MAJOR: q = k*32 + i]
B_TOTAL = 64
N_CORES = 8
B_PER_CORE = 8
GB = 4  # batches per partition-group
GROUPS = B_PER_CORE // GB  # 2
EPS = 1e-7
ROUTINGS = 3
NR_ITERS = 1  # Newton iterations for the quake rsqrt (1 -> ~0.2% max err)


def build_program():
    nc = bacc.Bacc("TRN2", target_bir_lowering=False, debug=False)

    # group-0 x, split for the two HWDGE rings (chunk-major so each chunk is
    # one contiguous-per-partition DMA)
    x_b0 = nc.declare_dram_parameter("x_b0", [4, 128, GB, 2, D_IN], BF16, isOutput=False)
    x_d0 = nc.declare_dram_parameter("x_d0", [2, 128, GB, 2, 512], F8, isOutput=False)
    # group-1 x, whole-tile partition-major via SWDGE
    x_b1 = nc.declare_dram_parameter("x_b1", [2, 128, GB, 4, D_IN], BF16, isOutput=False)
    x_d1 = nc.declare_dram_parameter("x_d1", [2, 128, GB, 2, 512], F8, isOutput=False)
    w_a = nc.declare_dram_parameter("w_a", [128, 2, IK], BF16, isOutput=False)
    w_t = nc.declare_dram_parameter("w_t", [128, 8, D_IN], BF16, isOutput=False)
    ident_d = nc.declare_dram_parameter("ident", [128, 128], BF16, isOutput=False)
    # idT[p, c] = (p%32 == c%32): every mask the kernel needs is a
    # broadcast-view of this one 32KB tile
    idT_d = nc.declare_dram_parameter("idT", [128, 128], BF16, isOutput=False)
    out_d = nc.declare_dram_parameter("out", [GROUPS, 128, DIM_CAPS], F32, isOutput=True)

    with ExitStack() as ctx:
        tc = ctx.enter_context(tile.TileContext(nc))
        singles = ctx.enter_context(tc.tile_pool(name="singles", bufs=1))
        xpool = ctx.enter_context(tc.tile_pool(name="xpool", bufs=2))
        work = ctx.enter_context(tc.tile_pool(name="work", bufs=3))
        psum = ctx.enter_context(tc.tile_pool(name="ps", bufs=1, space="PSUM"))

        w_a_sb = singles.tile([128, 2, IK], BF16)  # [d%128, d//128, q]
        w_t_sb = singles.tile([128, 8, D_IN], BF16)  # [q%128, q//128, d]
        ident_sb = singles.tile([128, 128], BF16)
        idT_sb = singles.tile([128, 128], BF16)
        xb_t = [
            xpool.tile([128, GB, 8, D_IN], BF16, tag="xb", name=f"xb{g}")
            for g in range(GROUPS)
        ]
        xd_t = [
            xpool.tile([128, GB, 2, 2, 512], F8, tag="xd", name=f"xd{g}")
            for g in range(GROUPS)
        ]

        # ---- DMA issue plan ----
        # The SWDGE queue hogs ~85% of HBM bandwidth once active, starving
        # the HWDGE rings.  So the rings carry ONLY group-0 xb (which fully
        # lands before SWDGE data starts flowing) + the tiny identity;
        # everything else rides SWDGE in deadline order.
        nc.sync.dma_start(out=xb_t[0][:, :, 0:2, :], in_=x_b0[0])
        nc.scalar.dma_start(out=xb_t[0][:, :, 2:4, :], in_=x_b0[1])
        nc.sync.dma_start(out=ident_sb[:, :], in_=ident_d[:, :])
        nc.scalar.dma_start(out=idT_sb[:, :], in_=idT_d[:, :])
        nc.gpsimd.dma_start(out=xb_t[0][:, :, 4:6, :], in_=x_b0[2])
        nc.gpsimd.dma_start(out=xb_t[0][:, :, 6:8, :], in_=x_b0[3])
        nc.gpsimd.dma_start(out=w_a_sb[:, :, :], in_=w_a[:, :, :])
        nc.gpsimd.dma_start(out=xd_t[0][:, :, :, 0, :], in_=x_d0[0])
        nc.gpsimd.dma_start(out=w_t_sb[:, :, :], in_=w_t[:, :, :])
        nc.gpsimd.dma_start(out=xb_t[1][:, :, 0:4, :], in_=x_b1[0])
        nc.gpsimd.dma_start(out=xb_t[1][:, :, 4:8, :], in_=x_b1[1])
        nc.gpsimd.dma_start(out=xd_t[0][:, :, :, 1, :], in_=x_d0[1])
        nc.gpsimd.dma_start(out=xd_t[1][:, :, :, 0, :], in_=x_d1[0])
        nc.gpsimd.dma_start(out=xd_t[1][:, :, :, 1, :], in_=x_d1[1])

        cu_sb = singles.tile([128, NUM_CAPS], BF16)
        nc.vector.memset(cu_sb[:, :], 1.0 / NUM_CAPS)
        magic_sb = singles.tile([128, 1], mybir.dt.int32)
        nc.vector.memset(magic_sb[:, :], 0x5F3759DF)
        one_i_sb = singles.tile([128, 1], mybir.dt.int32)
        nc.vector.memset(one_i_sb[:, :], 1)

        def rsqrt_dve(a_ap, tagp, v=None, nr_iters=NR_ITERS):
            """1/sqrt(a): quake bit-trick + Newton (ACT Rsqrt would thrash
            the Exp activation-table set)."""
            v = v or nc.vector
            t_i = work.tile([128, 1], mybir.dt.int32, tag=tagp + "i", name="nr_i")
            nc.vector.tensor_tensor(
                t_i[:, :], a_ap.bitcast(mybir.dt.int32), one_i_sb[:, :], ALU.logical_shift_right
            )
            r = work.tile([128, 1], F32, tag=tagp + "r", name="nr_r")
            nc.vector.tensor_tensor(
                r[:, :].bitcast(mybir.dt.int32), magic_sb[:, :], t_i[:, :], ALU.subtract
            )
            t2 = work.tile([128, 1], F32, tag=tagp + "t", name="nr_t")
            for _ in range(nr_iters):
                v.tensor_mul(t2[:, :], a_ap, r[:, :])
                v.tensor_mul(t2[:, :], t2[:, :], r[:, :])
                v.tensor_scalar(t2[:, :], t2[:, :], -0.5, 1.5, ALU.mult, ALU.add)
                v.tensor_mul(r[:, :], r[:, :], t2[:, :])
            return r

        def group_stream(g):
            xb = xb_t[g]
            xd = xd_t[g]
            store_eng = [nc.sync, nc.scalar][g]
            yield

            cT_sb = None  # [j%128, jc, (b,i)] softmax'd coupling coeffs
            for it in range(ROUTINGS):
                last = it == ROUTINGS - 1

                # ---- y-MM: y[(b,i), d] = sum_j c[(b,i), j] x[b][j, d] ----
                # group 0 iter 0 runs in chunk-arrival wave order so compute
                # starts as soon as the first HWDGE chunk lands.
                y4_ps = psum.tile([128, D_IN], F32, tag="m32", bufs=2, name="y4_ps")
                jc_order = tuple(range(8))
                for jc in jc_order:
                    for b in range(GB):
                        lhsT = cu_sb[:, :] if it == 0 else cT_sb[:, jc, 32 * b : 32 * b + 32]
                        nc.tensor.matmul(
                            y4_ps[32 * b : 32 * b + 32, :],
                            lhsT,
                            xb[:, b, jc, :],
                            start=(jc == 0),
                            stop=(jc == 7),
                            tile_position=(0, 32 * b),
                            skip_group_check=True,
                        )
                yield

                # evacuate (ACT) + PE-transpose y -> [d, (b,i)]
                y4_sb = work.tile([128, D_IN], BF16, tag="y4sb", name="y4_sb")
                nc.scalar.copy(y4_sb[:, :], y4_ps[:, :])
                yT_ps = psum.tile([128, 2, 128], BF16, tag="tp", bufs=2, name="yT_ps")
                for t in range(2):
                    nc.tensor.transpose(yT_ps[:, t, :], y4_sb[:, 128 * t : 128 * t + 128], ident_sb[:, :])
                yT_sb = work.tile([128, 2, 128], BF16, tag="yTsb", name="yT_sb")
                nc.scalar.copy(yT_sb[:, :, :], yT_ps[:, :, :])
                yield

                # ---- s-MM (cross): sc[(b,i), q] = sum_d y[(b,i),d] W[d,q],
                # masked to the block-diagonal during PSUM evacuation ----
                m4_sb = work.tile([128, IK], BF16, tag="m4", name="m4_sb")
                for nh in range(2):
                    sc_ps = psum.tile([128, 512], F32, tag="half", bufs=3, name="sc_ps")
                    for dc in range(2):
                        nc.tensor.matmul(
                            sc_ps[:, :],
                            yT_sb[:, dc, :],
                            w_a_sb[:, dc, 512 * nh : 512 * nh + 512],
                            start=(dc == 0),
                            stop=(dc == 1),
                            skip_group_check=True,
                        )
                    nc.vector.tensor_mul(
                        m4_sb[:, 512 * nh : 512 * nh + 512].rearrange(
                            "p (k i) -> p k i", i=NUM_CAPS
                        ),
                        sc_ps[:, :].rearrange("p (k i) -> p k i", i=NUM_CAPS),
                        idT_sb[:, 0:NUM_CAPS].unsqueeze(1).broadcast_to([128, 16, NUM_CAPS]),
                    )
                yield

                if last:
                    # compact s[(b,i), k] = sum_i' masked[(b,i), (k,i')]
                    # (contiguous last-axis reduce thanks to k-major W)
                    s4c = work.tile([128, DIM_CAPS], BF16, tag="s4c", name="s4c")
                    with nc.allow_low_precision(
                        reason="diag-select: 31 of 32 summands are exact zeros"
                    ):
                        # free-axis reduce is DVE-only (Pool only reduces C)
                        for h in range(2):
                            nc.vector.tensor_reduce(
                                s4c[:, 16 * h : 16 * h + 16],
                                m4_sb[:, 512 * h : 512 * h + 512].rearrange(
                                    "p (k i) -> p k i", i=NUM_CAPS
                                ),
                                axis=mybir.AxisListType.X,
                                op=ALU.add,
                            )
                    # per-half Square+accum: half 0's square runs on ACT
                    # while DVE still reduces half 1
                    sq_s = work.tile([128, DIM_CAPS], F32, tag="sqs", name="sq_s")
                    nsq_h = work.tile([128, 2], F32, tag="nsqh", name="nsq_h")
                    for h in range(2):
                        nc.scalar.activation(
                            sq_s[:, 16 * h : 16 * h + 16],
                            s4c[:, 16 * h : 16 * h + 16],
                            AF.Square,
                            accum_out=nsq_h[:, h : h + 1],
                        )
                    nsq = work.tile([128, 1], F32, tag="nsq", name="nsq")
                    tail_eng = nc.gpsimd if g == 0 else nc.vector
                    tail_eng.tensor_tensor(nsq[:, :], nsq_h[:, 0:1], nsq_h[:, 1:2], ALU.add)
                    tail_eng.tensor_scalar(nsq[:, :], nsq[:, :], EPS, None, ALU.add)
                    rn = rsqrt_dve(nsq[:, :], "lst", v=tail_eng)
                    o_out = work.tile([128, DIM_CAPS], F32, tag="oout", name="o_out")
                    tail_eng.tensor_scalar(o_out[:, :], s4c[:, :], rn[:, :], None, ALU.mult)
                    store_eng.dma_start(out=out_d[g], in_=o_out[:, :])
                    return

                # ---- squash norm (sum of squares over free dim) ----
                sq_scr = work.tile([128, IK], BF16, tag="scr", name="sq_scr")
                nsq4 = work.tile([128, 1], F32, tag="nsq4", name="nsq4")
                nc.scalar.activation(sq_scr[:, :], m4_sb[:, :], AF.Square, accum_out=nsq4[:, :])
                nc.vector.tensor_scalar(nsq4[:, :], nsq4[:, :], EPS, None, ALU.add)
                rn4 = rsqrt_dve(nsq4[:, :], "mid", v=nc.gpsimd, nr_iters=0)
                yield

                # ---- O[q,(b,i)] = m4 transposed: the masked cross product
                # already exists in SBUF, so 8 PE transposes replace the
                # 16-matmul W^T recompute AND both DVE mask-multiplies ----
                o_sb = work.tile([128, 8, 128], BF16, tag="osb", name="o_sb")
                oT_ps = psum.tile([128, 8, 128], BF16, tag="scTh", bufs=1, name="oT_ps")
                for q in range(8):
                    nc.tensor.transpose(
                        oT_ps[:, q, :], m4_sb[:, 128 * q : 128 * q + 128], ident_sb[:, :]
                    )
                nc.vector.tensor_copy(o_sb[:, 0:4, :], oT_ps[:, 0:4, :])
                nc.scalar.copy(o_sb[:, 4:8, :], oT_ps[:, 4:8, :])
                # ---- wt-MM: wt[d, (b,i)] = sum_q W[d,q] O[q,(b,i)] ----
                # NOTE: dc must stay the OUTER loop: start=True clears the
                # has_written bits of the whole PSUM bank, so the two dc
                # accumulation chains (same bank) must not interleave.
                wt_ps = psum.tile([128, 2, 128], F32, tag="m32", bufs=2, name="wt_ps")
                for dc in range(2):
                    for ikc in range(8):
                        nc.tensor.matmul(
                            wt_ps[:, dc, :],
                            w_t_sb[:, ikc, 128 * dc : 128 * dc + 128],
                            o_sb[:, ikc, :],
                            start=(ikc == 0),
                            stop=(ikc == 7),
                            skip_group_check=True,
                        )
                yield

                # evacuate (squash scale is folded into the Exp below)
                wt_sb = work.tile([128, 2, 128], BF16, tag="wtsb", name="wt_sb")
                nc.scalar.copy(wt_sb[:, :, :], wt_ps[:, :, :])
                yield

                # ---- b-MM + softmax, pipelined per j-half ----
                # blogit[(b,i), j] = sum_d wt[d,(b,i)] x[b][d, j]; then
                # e = Exp(rn4 * blogit) on ACT, PE-transpose e -> PSUM,
                # softmax-sum + normalize straight out of PSUM.
                cT_sb = work.tile([128, 8, 128], BF16, tag="cT", name="cT_sb")
                for jh in range(2):
                    b4_ps = psum.tile([128, 512], F32, tag="half", bufs=3, name="b4_ps")
                    for dc in range(2):
                        for b in range(GB):
                            nc.tensor.matmul(
                                b4_ps[32 * b : 32 * b + 32, :],
                                wt_sb[:, dc, 32 * b : 32 * b + 32],
                                xd[:, b, dc, jh, :],
                                start=(dc == 0),
                                stop=(dc == 1),
                                tile_position=(0, 32 * b),
                                skip_group_check=True,
                            )
                    # softmax per half: DVE does only the z-reduce (from
                    # PSUM); ACT evacuates eT in parallel; Pool normalizes
                    # with a single fused divide (no reciprocal op).
                    e4_sb = work.tile([128, 512], BF16, tag="e4", name="e4_sb")
                    eT_ps = psum.tile([128, 4, 128], BF16, tag="tp", bufs=2, name="eT_ps")
                    nc.scalar.activation(e4_sb[:, :], b4_ps[:, :], AF.Exp, scale=rn4[:, :])
                    for q in range(4):
                        nc.tensor.transpose(
                            eT_ps[:, q, :], e4_sb[:, 128 * q : 128 * q + 128], ident_sb[:, :]
                        )
                    zT_sb = work.tile([128, 4, GB], F32, tag="zT", name="zT_sb")
                    nc.vector.tensor_reduce(
                        zT_sb[:, :, :],
                        eT_ps[:, :, :].rearrange("p c (b i) -> p c b i", b=GB),
                        axis=mybir.AxisListType.X,
                        op=ALU.add,
                    )
                    rz_sb = work.tile([128, 4, GB], F32, tag="rz", name="rz_sb")
                    nc.vector.reciprocal(rz_sb[:, :, :], zT_sb[:, :, :])
                    nc.vector.tensor_tensor(
                        cT_sb[:, 4 * jh : 4 * jh + 4, :].rearrange(
                            "p c (b i) -> p c b i", b=GB
                        ),
                        eT_ps[:, :, :].rearrange("p c (b i) -> p c b i", b=GB),
                        rz_sb[:, :, :].unsqueeze(3).broadcast_to([128, 4, GB, NUM_CAPS]),
                        ALU.mult,
                    )
                    yield

        streams = [group_stream(g) for g in range(GROUPS)]
        alive = list(streams)
        rounds = 0
        while alive:
            rounds += 1
            if rounds == 16 and streams[0] in alive:
                # g1 lags g0 by ~1 DMA-bound iteration; emit g0's final
                # block early so its DVE tail isn't queued behind g1's
                # (time-wise later) softmax chain
                for _ in range(2):
                    try:
                        next(streams[0])
                    except StopIteration:
                        alive = [s for s in alive if s is not streams[0]]
                        break
            keep = []
            for s in alive:
                try:
                    next(s)
                    keep.append(s)
                except StopIteration:
                    pass
            alive = keep

    nc.compile()
    return nc


def _host_inputs(x, W):
    import ml_dtypes

    bf16 = ml_dtypes.bfloat16
    x = np.asarray(x, dtype=np.float32)
    W = np.asarray(W, dtype=np.float32).reshape(D_IN, IK)
    # k-major flattening: w_km[d, k*32 + i] = W[d, i*32 + k]
    w_km = np.ascontiguousarray(
        W.reshape(D_IN, NUM_CAPS, DIM_CAPS).transpose(0, 2, 1).reshape(D_IN, IK)
    )
    pp = np.arange(128)
    col = np.arange(128)
    # idT[p, c] = (p%32 == c%32) -- broadcast on chip into every mask
    idT = (pp[:, None] % NUM_CAPS == col[None, :] % NUM_CAPS).astype(bf16)
    # per-core layouts
    xc = x.reshape(N_CORES, B_PER_CORE, N_IN, D_IN)
    # group-0 xb: chunk-major [core, c, p=j%128 (within chunk pair), b, jj, d]
    # where j = (2c+jj)*128 + p
    x0 = xc[:, :GB].reshape(N_CORES, GB, 4, 2, 128, D_IN)  # [core, b, c, jj, p, d]
    xb0 = np.ascontiguousarray(x0.transpose(0, 2, 4, 1, 3, 5)).astype(bf16)
    # xd: fp8, jh-major halves [core, jh, p=d%128, b, dc, 512]
    f8 = ml_dtypes.float8_e4m3
    xT0 = xc[:, :GB].transpose(0, 1, 3, 2)  # [core, b, d, j]
    xd0 = np.ascontiguousarray(
        xT0.reshape(N_CORES, GB, 2, 128, 2, 512).transpose(0, 4, 3, 1, 2, 5)
    ).astype(f8)
    # group-1 xb: jc-half-major [core, h, p=j%128, b, jc%4, d]
    xb1 = np.ascontiguousarray(
        xc[:, GB:].reshape(N_CORES, GB, 2, 4, 128, D_IN).transpose(0, 2, 4, 1, 3, 5)
    ).astype(bf16)
    xT1 = xc[:, GB:].transpose(0, 1, 3, 2)  # [core, b, d, j]
    xd1 = np.ascontiguousarray(
        xT1.reshape(N_CORES, GB, 2, 128, 2, 512).transpose(0, 4, 3, 1, 2, 5)
    ).astype(f8)
    wa = np.ascontiguousarray(w_km.reshape(2, 128, IK).transpose(1, 0, 2)).astype(bf16)
    wt = np.ascontiguousarray(w_km.T.reshape(8, 128, D_IN).transpose(1, 0, 2)).astype(bf16)
    ident = np.eye(128, dtype=np.float32).astype(bf16)
    return xb0, xd0, xb1, xd1, wa, wt, idT, ident


_prog_cache = {}


def _get_program():
    if "nc" not in _prog_cache:
        _prog_cache["nc"] = build_program()
    return _prog_cache["nc"]


def make_in_maps(xb0, xd0, xb1, xd1, wa, wt, idT, ident):
    in_maps = []
    for c in range(N_CORES):
        in_maps.append(
            {
                "x_b0": xb0[c],
                "x_d0": xd0[c],
                "x_b1": xb1[c],
                "x_d1": xd1[c],
                "w_a": wa,
                "w_t": wt,
                "idT": idT,
                "ident": ident,
            }
        )
    return in_maps


def kernel(x, W):
    nc = _get_program()
    in_maps = make_in_maps(*_host_inputs(x, W))
    res = run_bass_kernel_spmd(nc, in_maps, core_ids=list(range(N_CORES)))
    out = np.empty((B_TOTAL, NUM_CAPS, DIM_CAPS), np.float32)
    for c in range(N_CORES):
        o = res.results[c]["out"]  # [GROUPS, 128, 32]; partition p = 32*b + i
        out[c * B_PER_CORE : (c + 1) * B_PER_CORE] = o.reshape(B_PER_CORE, NUM_CAPS, DIM_CAPS)
    return out
